# revision 7
# baseline (speedup 1.0000x reference)
"""DiffPool4GraphLayer Trainium2 kernel (8-core SPMD, data-parallel over graphs)."""
import os
import numpy as np

import concourse.bass as bass
import concourse.bacc as bacc
import concourse.mybir as mybir
import concourse.tile as tile
from concourse.bass_utils import run_bass_kernel_spmd

# ---- problem constants (hardcoded per harness contract) ----
N_GRAPHS = 32
NODES = 500          # real nodes per graph
NPAD = 512           # padded nodes per graph
CPG = 50             # clusters per graph
IN_DIM = 256
OUT_DIM = 256
ASSIGN = N_GRAPHS * CPG        # 1600
N = N_GRAPHS * NODES           # 16000
NPT = N_GRAPHS * NPAD          # 16384 padded
E = N * 16                     # 256000
NCORES = 8
GPC = N_GRAPHS // NCORES       # 4 graphs per core
VPC = GPC * NPAD               # 2048 padded nodes per core
TPC = VPC // 128               # 16 node-tiles per core
P = 128

f32 = mybir.dt.float32
f32r = mybir.dt.float32r
i32 = mybir.dt.int32

# dtype knob for matmul inputs: "f32" (exact) or "f32r" (4x faster, ~2e-4 rel)
MM_MODE = os.environ.get("KERNEL_MM", "f32")

LAST_RESULTS = []   # BassKernelResults of the last kernel() call (for profiling)
LAST_PROGRAMS = []  # finalized Bacc programs of the last kernel() call


def _pack_lanes(vals, nkt, dtype):
    """[nkt*128] lane-major -> [128, nkt] (partition p, ktile k)."""
    return np.ascontiguousarray(vals.reshape(nkt, P).T).astype(dtype)


def _ceil(a, b):
    return -(-a // b)


# ============================================================ host prep

def _prep(h, W_feat, b_feat, W_pool, b_pool, src, dst):
    src = src.astype(np.int64)
    dst = dst.astype(np.int64)
    g_src = src // NODES
    g_dst = dst // NODES
    src_p = (g_src * NPAD + src % NODES).astype(np.int32)
    dst_p = (g_dst * NPAD + dst % NODES).astype(np.int32)

    h_pad = np.zeros((NPT, IN_DIM), np.float32)
    h_pad.reshape(N_GRAPHS, NPAD, IN_DIM)[:, :NODES] = h.reshape(N_GRAPHS, NODES, IN_DIM)

    deg = np.bincount(dst, minlength=N).astype(np.float32)
    inv_deg = 1.0 / np.maximum(deg, 1.0)
    ew_edge = inv_deg[dst].astype(np.float32)

    core_of_edge = g_dst // GPC

    # ---------- launch 1 edge layout: per core, edges sorted by dst_p,
    # grouped per 128-node dst tile (16 slots/core), padded to shared ktc ----
    per_core = []
    for c in range(NCORES):
        m = core_of_edge == c
        e_idx = np.nonzero(m)[0]
        order = np.argsort(dst_p[e_idx], kind="stable")
        e_idx = e_idx[order]
        tile_id = (dst_p[e_idx] - c * VPC) // P      # 0..15
        cnt = np.bincount(tile_id, minlength=TPC)
        starts = np.concatenate([[0], np.cumsum(cnt)])
        per_core.append((e_idx, cnt, starts))

    ktc = [max(_ceil(int(per_core[c][1][t]), P) for c in range(NCORES))
           for t in range(TPC)]
    kofs = np.concatenate([[0], np.cumsum(ktc)]).astype(int)
    nkt1 = int(kofs[-1])

    l1 = []
    for c in range(NCORES):
        e_idx, cnt, starts = per_core[c]
        src_l = np.zeros(nkt1 * P, np.int32)
        dl_l = np.full(nkt1 * P, -1.0, np.float32)
        ew_l = np.zeros(nkt1 * P, np.float32)
        for t in range(TPC):
            n = int(cnt[t])
            sl = e_idx[starts[t]:starts[t] + n]
            o = kofs[t] * P
            src_l[o:o + n] = src_p[sl]
            dl_l[o:o + n] = (dst_p[sl] - (c * VPC + t * P)).astype(np.float32)
            ew_l[o:o + n] = ew_edge[sl]
        l1.append((_pack_lanes(src_l, nkt1, np.int32),
                   _pack_lanes(dl_l, nkt1, np.float32),
                   _pack_lanes(ew_l, nkt1, np.float32)))

    # ---------- launch 2 edge layout: per core, edges grouped by
    # (g_dst_local 0..3, g_src 0..31), padded to shared group k-counts ----
    key = (g_dst % GPC) * N_GRAPHS + g_src
    per_core2 = []
    for c in range(NCORES):
        m = core_of_edge == c
        e_idx = np.nonzero(m)[0]
        order = np.argsort(key[e_idx], kind="stable")
        e_idx = e_idx[order]
        cnt = np.bincount(key[e_idx], minlength=GPC * N_GRAPHS)
        starts = np.concatenate([[0], np.cumsum(cnt)])
        per_core2.append((e_idx, cnt, starts))

    gkt = [max(_ceil(int(per_core2[c][1][q]), P) for c in range(NCORES))
           for q in range(GPC * N_GRAPHS)]
    gofs = np.concatenate([[0], np.cumsum(gkt)]).astype(int)
    nkt2 = int(gofs[-1])

    PAD_IDX = NODES  # padded-dummy node of graph 0 -> s row is zero
    l2 = []
    for c in range(NCORES):
        e_idx, cnt, starts = per_core2[c]
        sd_l = np.full(nkt2 * P, PAD_IDX, np.int32)
        ss_l = np.full(nkt2 * P, PAD_IDX, np.int32)
        for q in range(GPC * N_GRAPHS):
            n = int(cnt[q])
            sl = e_idx[starts[q]:starts[q] + n]
            o = gofs[q] * P
            sd_l[o:o + n] = dst_p[sl]
            ss_l[o:o + n] = src_p[sl]
        l2.append((_pack_lanes(sd_l, nkt2, np.int32),
                   _pack_lanes(ss_l, nkt2, np.int32)))

    # ---------- per-core weight permutation: own 4 graphs' 50-col blocks first
    Wp_perm, col_perm = [], []
    for c in range(NCORES):
        own = np.arange(4 * c * CPG, (4 * c + GPC) * CPG)
        rest = np.setdiff1d(np.arange(ASSIGN), own)
        perm = np.concatenate([own, rest])
        col_perm.append(perm)
        Wp_perm.append(np.ascontiguousarray(W_pool[:, perm]))

    # h_ownT per core: [2, 128, VPC]
    hT = []
    for c in range(NCORES):
        hw = h_pad[c * VPC:(c + 1) * VPC].T  # [256, VPC]
        hT.append(np.ascontiguousarray(hw.reshape(2, P, VPC)).astype(np.float32))

    vmask = np.zeros((P, TPC), np.float32)
    for t in range(TPC):
        loc = (t % 4) * P + np.arange(P)
        vmask[:, t] = (loc < NODES).astype(np.float32)

    iota = np.tile(np.arange(P, dtype=np.float32), (P, 1))
    bp_perm = [np.ascontiguousarray(b_pool[col_perm[c]])[None, :] for c in range(NCORES)]

    return dict(h_pad=h_pad, hT=hT, l1=l1, l2=l2, ktc=ktc, kofs=kofs,
                gkt=gkt, gofs=gofs, nkt1=nkt1, nkt2=nkt2, Wp_perm=Wp_perm,
                col_perm=col_perm, vmask=vmask, iota=iota,
                bf=b_feat[None, :].astype(np.float32), bp_perm=bp_perm)


# ============================================================ launch 1 program

def _mmcast(pool, nc, ap, tag):
    """Optionally round an operand to f32r via DVE copy."""
    if MM_MODE != "f32r":
        return ap
    t = pool.tile(list(ap.shape), f32r, tag=tag, name=tag)
    nc.vector.tensor_copy(out=t[:], in_=ap)
    return t[:]


def _build_launch1(ktc, kofs, nkt1):
    nc = bacc.Bacc("TRN2", target_bir_lowering=False, debug=False,
                   num_devices=NCORES)
    t_hpad = nc.dram_tensor("h_pad", [NPT, IN_DIM], f32, kind="ExternalInput").ap()
    t_hT = nc.dram_tensor("hT", [2, P, VPC], f32, kind="ExternalInput").ap()
    t_wf = nc.dram_tensor("W_feat", [2 * IN_DIM, OUT_DIM], f32, kind="ExternalInput").ap()
    t_wp = nc.dram_tensor("W_pool", [2 * IN_DIM, ASSIGN], f32, kind="ExternalInput").ap()
    t_bf = nc.dram_tensor("b_feat", [1, OUT_DIM], f32, kind="ExternalInput").ap()
    t_bp = nc.dram_tensor("b_pool", [1, ASSIGN], f32, kind="ExternalInput").ap()
    t_src = nc.dram_tensor("src2d", [P, nkt1], i32, kind="ExternalInput").ap()
    t_dl = nc.dram_tensor("dl2d", [P, nkt1], f32, kind="ExternalInput").ap()
    t_ew = nc.dram_tensor("ew2d", [P, nkt1], f32, kind="ExternalInput").ap()
    t_vm = nc.dram_tensor("vmask", [P, TPC], f32, kind="ExternalInput").ap()
    t_iota = nc.dram_tensor("iota", [P, P], f32, kind="ExternalInput").ap()
    o_s = nc.dram_tensor("s_out", [VPC, CPG], f32, kind="ExternalOutput").ap()
    o_hp = nc.dram_tensor("hp_out", [GPC * CPG, OUT_DIM], f32, kind="ExternalOutput").ap()

    NC1 = [512, 512, 512, 64]   # pool bundle N-chunks
    with tile.TileContext(nc) as tc:
        with (
            tc.tile_pool(name="cst", bufs=1) as cst,
            tc.tile_pool(name="sb", bufs=3) as sb,
            tc.tile_pool(name="ev", bufs=2) as ev,
            tc.tile_pool(name="psA", bufs=1, space="PSUM") as psA,
            tc.tile_pool(name="psB", bufs=1, space="PSUM") as psB,
            tc.tile_pool(name="pp", bufs=1, space="PSUM") as pp,
        ):
            iota_sb = cst.tile([P, P], f32, tag="iota")
            nc.sync.dma_start(out=iota_sb[:], in_=t_iota[:])
            vm_sb = cst.tile([P, TPC], f32, tag="vm")
            nc.sync.dma_start(out=vm_sb[:], in_=t_vm[:])
            src_sb = cst.tile([P, nkt1], i32, tag="src")
            nc.sync.dma_start(out=src_sb[:], in_=t_src[:])
            dl_sb = cst.tile([P, nkt1], f32, tag="dl")
            nc.sync.dma_start(out=dl_sb[:], in_=t_dl[:])
            ew_sb = cst.tile([P, nkt1], f32, tag="ew")
            nc.sync.dma_start(out=ew_sb[:], in_=t_ew[:])
            hT_sb = [cst.tile([P, VPC], f32, tag=f"hT{k}", name=f"hT{k}") for k in range(2)]
            for k in range(2):
                nc.sync.dma_start(out=hT_sb[k][:], in_=t_hT[k])
            wf_sb = [cst.tile([P, OUT_DIM], f32, tag=f"wf{k}", name=f"wf{k}") for k in range(4)]
            wp_sb = [cst.tile([P, ASSIGN], f32, tag=f"wp{k}", name=f"wp{k}") for k in range(4)]
            for k in range(4):
                nc.sync.dma_start(out=wf_sb[k][:], in_=t_wf[k * P:(k + 1) * P, :])
                nc.sync.dma_start(out=wp_sb[k][:], in_=t_wp[k * P:(k + 1) * P, :])
            ones_sb = cst.tile([1, P], f32, tag="ones")
            nc.vector.memset(ones_sb[:], 1.0)
            bf_sb = cst.tile([1, OUT_DIM], f32, tag="bf")
            nc.sync.dma_start(out=bf_sb[:], in_=t_bf[:])
            bp_sb = cst.tile([1, ASSIGN], f32, tag="bp")
            nc.sync.dma_start(out=bp_sb[:], in_=t_bp[:])

            if MM_MODE == "f32r":
                for k in range(2):
                    hT_sb[k] = _roundr(nc, cst, hT_sb[k], f"hTr{k}")
                for k in range(4):
                    wf_sb[k] = _roundr(nc, cst, wf_sb[k], f"wfr{k}")
                    wp_sb[k] = _roundr(nc, cst, wp_sb[k], f"wpr{k}")

            feat_all = cst.tile([P, TPC * OUT_DIM], f32, tag="feat_all")
            s_all = cst.tile([P, TPC * CPG], f32, tag="s_all")

            for t in range(TPC):
                # ---- aggregation: cT[feat, node] via indicator matmuls ----
                cT_ps = psA.tile([P, 1024], f32, tag="cT_ps")  # halves bank-separated (start=True clears whole-bank has_written)
                nk = ktc[t]
                for j in range(nk):
                    k = int(kofs[t]) + j
                    g = sb.tile([P, IN_DIM], f32, tag="gath")
                    nc.gpsimd.indirect_dma_start(
                        out=g[:], out_offset=None, in_=t_hpad[:],
                        in_offset=bass.IndirectOffsetOnAxis(
                            ap=src_sb[:, k:k + 1], axis=0))
                    ind = sb.tile([P, P], f32 if MM_MODE != "f32r" else f32r,
                                  tag="ind")
                    nc.vector.tensor_scalar(
                        out=ind[:], in0=iota_sb[:], scalar1=dl_sb[:, k:k + 1],
                        scalar2=ew_sb[:, k:k + 1],
                        op0=mybir.AluOpType.is_equal, op1=mybir.AluOpType.mult)
                    if MM_MODE == "f32r":
                        gr = sb.tile([P, IN_DIM], f32r, tag="gathr")
                        nc.vector.tensor_copy(out=gr[:], in_=g[:])
                        g = gr
                    for half in range(2):
                        nc.tensor.matmul(
                            out=cT_ps[:, half * 512:half * 512 + P],
                            lhsT=g[:, half * P:(half + 1) * P], rhs=ind[:],
                            start=(j == 0), stop=(j == nk - 1))
                cT_sb = ev.tile([P, 2 * P], f32 if MM_MODE != "f32r" else f32r,
                                tag="cT_sb")
                nc.vector.tensor_copy(out=cT_sb[:, 0:P], in_=cT_ps[:, 0:P])
                nc.vector.tensor_copy(out=cT_sb[:, P:2 * P], in_=cT_ps[:, 512:512 + P])

                xT = [hT_sb[0][:, t * P:(t + 1) * P],
                      hT_sb[1][:, t * P:(t + 1) * P],
                      cT_sb[:, 0:P], cT_sb[:, P:2 * P]]

                # ---- feat bundle ----
                bf_ps = psB.tile([P, OUT_DIM], f32, tag="bf_ps")
                for k in range(4):
                    nc.tensor.matmul(out=bf_ps[:], lhsT=xT[k], rhs=wf_sb[k][:],
                                     start=(k == 0), stop=False)
                nc.tensor.matmul(out=bf_ps[:], lhsT=ones_sb[:], rhs=bf_sb[:],
                                 start=False, stop=True)

                # ---- pool bundle, 4 N-chunks ----
                bp_big = psB.tile([P, 2048], f32, tag="bp_big")
                bp_ps = [bp_big[:, n * 512:n * 512 + NC1[n]] for n in range(4)]
                for n in range(4):
                    lo = sum(NC1[:n])
                    sl = slice(lo, lo + NC1[n])
                    for k in range(4):
                        nc.tensor.matmul(out=bp_ps[n], lhsT=xT[k],
                                         rhs=wp_sb[k][:, sl],
                                         start=(k == 0), stop=False)
                    nc.tensor.matmul(out=bp_ps[n], lhsT=ones_sb[:],
                                     rhs=bp_sb[:, sl], start=False, stop=True)

                # ---- norms ----
                scr = ev.tile([P, 512], f32, tag="scr")
                ssf = ev.tile([P, 1], f32, tag="ssf")
                nc.scalar.activation(out=scr[:, :OUT_DIM], in_=bf_ps[:],
                                     func=mybir.ActivationFunctionType.Square,
                                     accum_out=ssf[:, 0:1])
                ssp = [ev.tile([P, 1], f32, tag=f"ssp{n}", name=f"ssp{n}") for n in range(4)]
                for n in range(4):
                    nc.scalar.activation(out=scr[:, :NC1[n]], in_=bp_ps[n],
                                         func=mybir.ActivationFunctionType.Square,
                                         accum_out=ssp[n][:, 0:1])
                nc.vector.tensor_add(out=ssp[0][:], in0=ssp[0][:], in1=ssp[1][:])
                nc.vector.tensor_add(out=ssp[2][:], in0=ssp[2][:], in1=ssp[3][:])
                nc.vector.tensor_add(out=ssp[0][:], in0=ssp[0][:], in1=ssp[2][:])

                def inv_norm(ss, tag):
                    nr = ev.tile([P, 1], f32, tag=tag, name=tag)
                    nc.scalar.activation(out=nr[:], in_=ss[:],
                                         func=mybir.ActivationFunctionType.Sqrt)
                    nc.vector.tensor_scalar_max(out=nr[:], in0=nr[:], scalar1=1e-12)
                    nc.vector.reciprocal(out=nr[:], in_=nr[:])
                    return nr

                invf = inv_norm(ssf, "invf")
                invp = inv_norm(ssp[0], "invp")

                # ---- feat = relu(bundle * invf) ----
                nc.scalar.activation(
                    out=feat_all[:, t * OUT_DIM:(t + 1) * OUT_DIM], in_=bf_ps[:],
                    func=mybir.ActivationFunctionType.Relu, scale=invf[:, 0:1])

                # ---- assign block -> softmax -> s ----
                gl = t // 4
                ablk = ev.tile([P, CPG], f32, tag="ablk")
                nc.scalar.activation(out=ablk[:], in_=bp_big[:, gl * CPG:(gl + 1) * CPG],
                                     func=mybir.ActivationFunctionType.Relu,
                                     scale=invp[:, 0:1])
                negmax = ev.tile([P, 1], f32, tag="negmax")
                nc.vector.tensor_reduce(out=negmax[:], in_=ablk[:],
                                        axis=mybir.AxisListType.X,
                                        op=mybir.AluOpType.max, negate=True)
                ex = ev.tile([P, CPG], f32, tag="ex")
                sumexp = ev.tile([P, 1], f32, tag="sumexp")
                nc.scalar.activation(out=ex[:], in_=ablk[:],
                                     func=mybir.ActivationFunctionType.Exp,
                                     bias=negmax[:, 0:1], accum_out=sumexp[:, 0:1])
                rec = ev.tile([P, 1], f32, tag="rec")
                nc.vector.tensor_scalar_add(out=rec[:], in0=sumexp[:], scalar1=1e-13)
                nc.vector.reciprocal(out=rec[:], in_=rec[:])
                nc.vector.tensor_tensor(out=rec[:], in0=rec[:],
                                        in1=vm_sb[:, t:t + 1],
                                        op=mybir.AluOpType.mult)
                nc.vector.tensor_scalar_mul(out=s_all[:, t * CPG:(t + 1) * CPG],
                                            in0=ex[:], scalar1=rec[:, 0:1])
                nc.sync.dma_start(out=o_s[t * P:(t + 1) * P, :],
                                  in_=s_all[:, t * CPG:(t + 1) * CPG])

            # ---- h_pool = s^T @ feat per graph ----
            for gl in range(GPC):
                hp_ps = pp.tile([CPG, OUT_DIM], f32, tag="hp_ps")
                for tt in range(4):
                    t = gl * 4 + tt
                    lhsT = s_all[:, t * CPG:(t + 1) * CPG]
                    rhs = feat_all[:, t * OUT_DIM:(t + 1) * OUT_DIM]
                    if MM_MODE == "f32r":
                        lr = ev.tile([P, CPG], f32r, tag="s_r")
                        nc.vector.tensor_copy(out=lr[:], in_=lhsT)
                        rr = ev.tile([P, OUT_DIM], f32r, tag="f_r")
                        nc.vector.tensor_copy(out=rr[:], in_=rhs)
                        lhsT, rhs = lr[:], rr[:]
                    nc.tensor.matmul(out=hp_ps[:], lhsT=lhsT, rhs=rhs,
                                     start=(tt == 0), stop=(tt == 3))
                hp_sb = ev.tile([CPG, OUT_DIM], f32, tag="hp_sb")
                nc.vector.tensor_copy(out=hp_sb[:], in_=hp_ps[:])
                nc.sync.dma_start(out=o_hp[gl * CPG:(gl + 1) * CPG, :], in_=hp_sb[:])

    nc.finalize()
    return nc


def _roundr(nc, pool, t_sb, tag):
    r = pool.tile(list(t_sb.shape), f32r, tag=tag, name=tag)
    nc.vector.tensor_copy(out=r[:], in_=t_sb[:])
    return r


# ============================================================ launch 2 program

def _build_launch2(gkt, gofs, nkt2):
    nc = bacc.Bacc("TRN2", target_bir_lowering=False, debug=False,
                   num_devices=NCORES)
    t_s = nc.dram_tensor("s_full", [NPT, CPG], f32, kind="ExternalInput").ap()
    t_sd = nc.dram_tensor("sd2d", [P, nkt2], i32, kind="ExternalInput").ap()
    t_ss = nc.dram_tensor("ss2d", [P, nkt2], i32, kind="ExternalInput").ap()
    o_adj = nc.dram_tensor("adj_out", [GPC * CPG, ASSIGN], f32, kind="ExternalOutput").ap()

    NBLK = 8
    with tile.TileContext(nc) as tc:
        with (
            tc.tile_pool(name="cst", bufs=1) as cst,
            tc.tile_pool(name="sb", bufs=6) as sb,
            tc.tile_pool(name="ev", bufs=2) as ev,
            tc.tile_pool(name="ps", bufs=2, space="PSUM") as ps,
        ):
            sd_idx = cst.tile([P, nkt2], i32, tag="sdidx")
            nc.sync.dma_start(out=sd_idx[:], in_=t_sd[:])
            ss_idx = cst.tile([P, nkt2], i32, tag="ssidx")
            nc.sync.dma_start(out=ss_idx[:], in_=t_ss[:])

            for gl in range(GPC):
                pa = [ps.tile([CPG, NBLK * CPG], f32, tag=f"pa{n}", name=f"pa{n}") for n in range(4)]
                for g2 in range(N_GRAPHS):
                    q = gl * N_GRAPHS + g2
                    n, off = g2 // NBLK, (g2 % NBLK) * CPG
                    nk = gkt[q]
                    for j in range(nk):
                        k = int(gofs[q]) + j
                        sd = sb.tile([P, CPG], f32, tag="sd")
                        nc.gpsimd.indirect_dma_start(
                            out=sd[:], out_offset=None, in_=t_s[:],
                            in_offset=bass.IndirectOffsetOnAxis(
                                ap=sd_idx[:, k:k + 1], axis=0))
                        ssrc = sb.tile([P, CPG], f32, tag="ssrc")
                        nc.gpsimd.indirect_dma_start(
                            out=ssrc[:], out_offset=None, in_=t_s[:],
                            in_offset=bass.IndirectOffsetOnAxis(
                                ap=ss_idx[:, k:k + 1], axis=0))
                        lhsT, rhs = sd[:], ssrc[:]
                        if MM_MODE == "f32r":
                            lr = sb.tile([P, CPG], f32r, tag="sdr")
                            nc.vector.tensor_copy(out=lr[:], in_=lhsT)
                            rr = sb.tile([P, CPG], f32r, tag="ssr")
                            nc.vector.tensor_copy(out=rr[:], in_=rhs)
                            lhsT, rhs = lr[:], rr[:]
                        nc.tensor.matmul(out=pa[n][:, off:off + CPG],
                                         lhsT=lhsT, rhs=rhs,
                                         start=(j == 0), stop=(j == nk - 1))
                adj_sb = ev.tile([CPG, ASSIGN], f32, tag="adj_sb")
                for n in range(4):
                    nc.vector.tensor_copy(
                        out=adj_sb[:, n * NBLK * CPG:(n + 1) * NBLK * CPG],
                        in_=pa[n][:])
                nc.sync.dma_start(out=o_adj[gl * CPG:(gl + 1) * CPG, :],
                                  in_=adj_sb[:])

    nc.finalize()
    return nc


# ============================================================ entry point

def kernel(h, W_feat, b_feat, W_pool, b_pool, src, dst):
    global LAST_RESULTS, LAST_PROGRAMS
    LAST_RESULTS = []
    LAST_PROGRAMS = []
    h = np.asarray(h, np.float32)
    W_feat = np.asarray(W_feat, np.float32)
    b_feat = np.asarray(b_feat, np.float32)
    W_pool = np.asarray(W_pool, np.float32)
    b_pool = np.asarray(b_pool, np.float32)
    src = np.asarray(src)
    dst = np.asarray(dst)

    pr = _prep(h, W_feat, b_feat, W_pool, b_pool, src, dst)
    trace = bool(os.environ.get("KERNEL_TRACE"))

    nc1 = _build_launch1(pr["ktc"], pr["kofs"], pr["nkt1"])
    in_maps1 = []
    for c in range(NCORES):
        s2, d2, e2 = pr["l1"][c]
        in_maps1.append({
            "h_pad": pr["h_pad"], "hT": pr["hT"][c],
            "W_feat": W_feat, "W_pool": pr["Wp_perm"][c],
            "b_feat": pr["bf"], "b_pool": pr["bp_perm"][c],
            "src2d": s2, "dl2d": d2, "ew2d": e2,
            "vmask": pr["vmask"], "iota": pr["iota"],
        })
    LAST_PROGRAMS.append(nc1)
    r1 = run_bass_kernel_spmd(nc1, in_maps1, list(range(NCORES)), trace=trace)
    LAST_RESULTS.append(r1)

    s_full = np.concatenate([r1.results[c]["s_out"] for c in range(NCORES)], axis=0)
    h_pool = np.concatenate([r1.results[c]["hp_out"] for c in range(NCORES)], axis=0)

    nc2 = _build_launch2(pr["gkt"], pr["gofs"], pr["nkt2"])
    in_maps2 = []
    for c in range(NCORES):
        sd2, ss2 = pr["l2"][c]
        in_maps2.append({"s_full": s_full, "sd2d": sd2, "ss2d": ss2})
    LAST_PROGRAMS.append(nc2)
    r2 = run_bass_kernel_spmd(nc2, in_maps2, list(range(NCORES)), trace=trace)
    LAST_RESULTS.append(r2)

    # adj rows of core c are for its graphs' clusters but with PERMUTED src
    # columns? no: launch2 used global g_src blocks -> columns are global.
    adj = np.concatenate([r2.results[c]["adj_out"] for c in range(NCORES)], axis=0)
    return adj.astype(np.float32), h_pool.astype(np.float32)
